# revision 2
# baseline (speedup 1.0000x reference)
"""CombinedRankingLoss Trainium2 Bass kernel (concatenated-scan version).

Data-parallel over 8 NeuronCores: each core takes a [1024, 1024] slice of
scores/labels, computes partial sums of the loss components, host combines
the 8 partial vectors into the final scalar.

Math (x = scores/clip(temp), b = labels>0, e = exp(x)):
  ListMLE with labels in {0..4}: for element j with label v>=1,
    T_j = D_v - P_exc,v(j)
  where P_exc,v = exclusive prefix of e*[l==v] along the row and
  D_v = sum of e over labels <= v.  Concatenating the four masked-e
  streams per row-group as [me_4 ++ me_3 ++ me_2 ++ me_1] and running ONE
  exclusive-prefix scan with initial = 1 - S gives, at class-v's segment,
    out = 1 - S + sum_{u>v} E_u + P_exc,v(j) = P_exc,v(j) - (D_v - 1)
  because S = D_v + sum_{u>v} E_u.  That is z_v for every class at once —
  no D-chain, 8 scans instead of 32.  Then uv = m_cat * z (one flat
  multiply; masks disjoint and {0,1}, pad lane 0) and, exactly,
    sum_j ln(1 - uv(j)) = sum_{j labeled} ln T_j
  so one ACT Ln(scale=-1, bias=1) with accum_out per row-group produces
  the per-group lnT sums over all four classes.
  per_list = (L_g - inv_t*Wx_g) / (K_g + eps); host sums over rows.
  Focal/BCE: ur = scob - 2*mpos*scob = x*(1-2b) in bf16;
  pneg = sigmoid(-inv_t*ur); s2 = (1-pneg)^2; F = sum s2*ln(pneg);
  C = sum ln(pneg); Ssco = sum x.  Host combines (A, F, C, Wx, Ssco).

Engine layout: DVE owns the scans, masks (tensor_scalar), and the
tensor_tensor multiplies; ACT does exp (+S accum), scob copy (+Ssco),
K via Sign (+accum), sigmoid/square/ln, and the per-group masked-ln
accums.  The focal chain issues before the scans so its ACT work
overlaps them.  GPSIMD measured far below roofline when interleaved
with this flow; unused.
"""

import numpy as np

import concourse.bass as bass
import concourse.bacc as bacc
import concourse.mybir as mybir
from concourse.tile import TileContext

AL = mybir.AluOpType
AF = mybir.ActivationFunctionType
AX = mybir.AxisListType
F32 = mybir.dt.float32
BF16 = mybir.dt.bfloat16
I32 = mybir.dt.int32

N_CORES = 8
B_FULL = 8192
N = 1024
ROWS_PER_CORE = B_FULL // N_CORES
EPS = 1e-10


def build_nc(rows=ROWS_PER_CORE, n=N, GS=2, time_reps=1):
    P = 128
    G = rows // P
    S_STEPS = G // GS
    W = 4 * n + 1  # concat width incl pad col

    nc = bacc.Bacc("TRN2", target_bir_lowering=False, debug=False)
    d_scores = nc.dram_tensor("scores", [rows, n], F32, kind="ExternalInput")
    d_labels = nc.dram_tensor("labels", [rows, n], I32, kind="ExternalInput")
    d_temp = nc.dram_tensor("temperature", [1], F32, kind="ExternalInput")
    d_out = nc.dram_tensor("out", [1, 8], F32, kind="ExternalOutput")

    sc_re = d_scores.rearrange("(g p) n -> p g n", p=P)
    lb_re = d_labels.rearrange("(g p) n -> p g n", p=P)
    tp_re = d_temp.rearrange("(p a) -> p a", p=1)

    def fl(t):
        return t[:].rearrange("p g n -> p (g n)")

    with TileContext(nc) as tc:
        with (
            tc.tile_pool(name="const", bufs=1) as cpool,
            tc.tile_pool(name="io", bufs=2) as iopool,
            tc.tile_pool(name="wk", bufs=1) as wk,
            tc.tile_pool(name="st", bufs=1) as st,
            tc.tile_pool(name="ps", bufs=1, space="PSUM") as pspool,
        ):
            ones_n = cpool.tile([P, 4 * n], BF16, tag="ones", name="ones")
            nc.vector.memset(ones_n[:], 1.0)
            t_raw = cpool.tile([P, 1], F32, tag="traw", name="traw")
            t_clip = cpool.tile([P, 1], F32, tag="tclip", name="tclip")
            inv_t = cpool.tile([P, 1], F32, tag="invt", name="invt")
            neg_inv_t = cpool.tile([P, 1], F32, tag="ninvt", name="ninvt")
            nc.sync.dma_start(t_raw[:], tp_re[:, :].partition_broadcast(P))
            nc.vector.tensor_scalar(t_clip[:], t_raw[:], 0.1, 5.0, AL.max, AL.min)
            nc.vector.reciprocal(inv_t[:], t_clip[:])
            nc.vector.tensor_scalar(neg_inv_t[:], inv_t[:], -1.0, None, AL.mult)

            tS = st.tile([P, G], F32, tag="S", name="S")
            tL = st.tile([P, G], F32, tag="L", name="L")
            tWx = st.tile([P, G], F32, tag="Wx", name="Wx")
            tK = st.tile([P, G], F32, tag="K", name="K")
            tF = st.tile([P, S_STEPS], F32, tag="F", name="F")
            tC = st.tile([P, S_STEPS], F32, tag="C", name="C")
            tX = st.tile([P, S_STEPS], F32, tag="X", name="X")
            for _t in (tS, tL, tWx, tK, tF, tC, tX):
                nc.vector.memset(_t[:], 0.0)

            from contextlib import nullcontext
            loop_cm = tc.For_i(0, time_reps, 1) if time_reps > 1 else nullcontext()
            with loop_cm:
              for s in range(S_STEPS):
                g0 = s * GS
                sl = slice(g0, g0 + GS)
                sco = iopool.tile([P, GS, n], F32, tag="sco", name="sco")
                lab = iopool.tile([P, GS, n], I32, tag="lab", name="lab")
                nc.sync.dma_start(sco[:], sc_re[:, sl, :])
                nc.sync.dma_start(lab[:], lb_re[:, sl, :])

                labf = wk.tile([P, GS, n], BF16, tag="labf", name="labf")
                e = wk.tile([P, GS, n], BF16, tag="e", name="e")
                m_cat = wk.tile([P, GS, W], BF16, tag="mcat", name="mcat")
                me_cat = wk.tile([P, GS, W], BF16, tag="mecat", name="mecat")
                z = wk.tile([P, GS, W], BF16, tag="z", name="z")
                negD4 = wk.tile([P, GS], F32, tag="negD4", name="negD4")
                scob = wk.tile([P, GS, n], BF16, tag="scob", name="scob")
                mpos = wk.tile([P, GS, n], BF16, tag="mpos", name="mpos")
                w_t = wk.tile([P, GS, n], BF16, tag="w", name="w")
                t1 = wk.tile([P, GS, n], BF16, tag="t1", name="t1")
                pneg = wk.tile([P, GS, n], BF16, tag="pneg", name="pneg")

                nc.vector.tensor_scalar(labf[:], lab[:], 1.0, None, AL.mult)
                # scob first: unblocks the DVE w-chain earliest
                nc.scalar.activation(fl(scob), fl(sco), AF.Copy,
                                     accum_out=tX[:, s : s + 1])
                for gi in range(GS):
                    nc.scalar.activation(
                        e[:, gi, :], sco[:, gi, :], AF.Exp,
                        scale=inv_t[:], accum_out=tS[:, g0 + gi : g0 + gi + 1])
                for gi in range(GS):
                    nc.scalar.activation(
                        pneg[:, gi, :], labf[:, gi, :], AF.Sign,
                        accum_out=tK[:, g0 + gi : g0 + gi + 1])
                # masks into concat segments (class 4 first); pad col -> 0
                nc.vector.memset(m_cat[:, :, 4 * n : W], 0.0)
                for seg, v in enumerate((4, 3, 2, 1)):
                    nc.vector.tensor_scalar(
                        m_cat[:, :, seg * n : (seg + 1) * n], labf[:],
                        float(v), None, AL.is_equal)
                nc.vector.tensor_scalar(fl(mpos), fl(labf), 1.0, None,
                                        AL.is_ge)
                # focal chain (its ACT part overlaps the scans below)
                nc.vector.tensor_tensor(fl(w_t), fl(mpos), fl(scob), AL.mult)
                nc.vector.tensor_reduce(tWx[:, sl], w_t[:], AX.X, AL.add)
                nc.vector.tensor_scalar(fl(t1), fl(w_t), -2.0, None, AL.mult)
                ur = w_t  # w dead after t1
                nc.vector.tensor_tensor(fl(ur), fl(t1), fl(scob), AL.add)
                nc.scalar.activation(fl(pneg), fl(ur), AF.Sigmoid,
                                     scale=neg_inv_t[:])
                s2 = t1  # t1 dead after ur
                nc.scalar.activation(fl(s2), fl(pneg), AF.Square,
                                     scale=-1.0, bias=1.0)
                lnp = mpos  # mpos dead after w
                nc.scalar.activation(fl(lnp), fl(pneg), AF.Ln,
                                     accum_out=tC[:, s : s + 1])
                # me_cat segments = m_seg * e
                for seg in range(4):
                    nc.vector.tensor_tensor(
                        me_cat[:, :, seg * n : (seg + 1) * n],
                        m_cat[:, :, seg * n : (seg + 1) * n], e[:], AL.mult)
                nc.vector.tensor_scalar(negD4[:], tS[:, sl], -1.0, 1.0,
                                        AL.mult, AL.add)
                # one scan per row-group over the 4n concat
                for g in range(GS):
                    nc.vector.tensor_tensor_scan(
                        z[:, g, 1:W], ones_n[:], me_cat[:, g, 0 : 4 * n],
                        negD4[:, g : g + 1], AL.mult, AL.add)
                nc.vector.tensor_copy(
                    z[:, :, 0:1], negD4[:].rearrange("p (g o) -> p g o", o=1))
                # gg here: sigmoid/square/ln surely done, DVE mid-flight
                gg = pneg  # pneg dead after s2/lnp
                nc.vector.tensor_tensor(fl(gg), fl(s2), fl(lnp), AL.mult)
                nc.vector.tensor_reduce(tF[:, s : s + 1], fl(gg), AX.X, AL.add)
                # uv = m_cat * z (one flat multiply), per-group masked-ln
                nc.vector.tensor_tensor(fl(me_cat), fl(m_cat), fl(z), AL.mult)
                for g in range(GS):
                    nc.scalar.activation(
                        z[:, g, :], me_cat[:, g, :], AF.Ln,
                        scale=-1.0, bias=1.0,
                        accum_out=tL[:, g0 + g : g0 + g + 1])

            # ---- epilogue ----
            ep = st.tile([P, G], F32, tag="ep1", name="ep1")
            ep2 = st.tile([P, G], F32, tag="ep2", name="ep2")
            stat = st.tile([P, 8], F32, tag="stat", name="stat")
            statr = st.tile([P, 8], F32, tag="statr", name="statr")
            nc.vector.memset(stat[:], 0.0)
            nc.vector.tensor_reduce(stat[:, 1:2], tF[:], AX.X, AL.add)
            nc.vector.tensor_reduce(stat[:, 2:3], tC[:], AX.X, AL.add)
            nc.vector.tensor_reduce(stat[:, 3:4], tWx[:], AX.X, AL.add)
            nc.vector.tensor_reduce(stat[:, 4:5], tX[:], AX.X, AL.add)
            nc.vector.tensor_scalar(ep2[:], tWx[:], inv_t[:], None, AL.mult)
            nc.vector.tensor_tensor(ep[:], tL[:], ep2[:], AL.subtract)
            nc.vector.tensor_scalar(ep2[:], tK[:], EPS, None, AL.add)
            nc.vector.reciprocal(ep2[:], ep2[:])
            nc.vector.tensor_tensor(ep[:], ep[:], ep2[:], AL.mult)
            nc.vector.tensor_reduce(stat[:, 0:1], ep[:], AX.X, AL.add)
            ones_f = cpool.tile([P, 1], F32, tag="onesf", name="onesf")
            nc.vector.memset(ones_f[:], 1.0)
            ps_out = pspool.tile([1, 8], F32, tag="psout", name="psout")
            nc.tensor.matmul(ps_out[:], ones_f[:], stat[:], start=True, stop=True)
            nc.vector.tensor_copy(statr[0:1, :], ps_out[:])
            nc.sync.dma_start(d_out[:, :], statr[0:1, :])
    nc.compile()
    return nc


def combine_partials(parts, temp_val, b_full, n):
    """parts: [n_cores, 8] f32 partial sums; cols: A, F, C, Wx, Ssco."""
    parts = np.asarray(parts, dtype=np.float64)
    inv_t = 1.0 / float(np.clip(temp_val, 0.1, 5.0))
    A = parts[:, 0].sum()      # sum of per-row listmle
    F_ = parts[:, 1].sum()     # sum s2*lnp
    C = parts[:, 2].sum()      # sum lnp
    Wx = parts[:, 3].sum()     # sum b*scores (bf16)
    Ss = parts[:, 4].sum()     # sum scores
    U = Ss - 2.0 * Wx          # sum scores*(1-2b)
    cnt = float(b_full) * n
    listmle = A / b_full
    ce_sum = -C
    focal = (0.25 * -F_) / cnt
    sum_x = inv_t * (U + 2.0 * Wx)
    sum_xb = inv_t * Wx
    smooth = (ce_sum + 0.1 * sum_xb - (0.1 / n) * sum_x) / cnt
    total = 0.7 * listmle + 0.3 * focal + 0.1 * smooth
    return np.asarray(total, dtype=np.float32)


_CACHED = {}


def kernel(scores, temperature, labels):
    from concourse.bass_utils import run_bass_kernel_spmd

    scores = np.ascontiguousarray(np.asarray(scores), dtype=np.float32)
    labels = np.ascontiguousarray(np.asarray(labels), dtype=np.int32)
    temperature = np.asarray(temperature, dtype=np.float32).reshape(1)

    key = (scores.shape, labels.shape)
    if key not in _CACHED:
        _CACHED[key] = build_nc(rows=scores.shape[0] // N_CORES,
                                n=scores.shape[1])
    nc = _CACHED[key]

    rows = scores.shape[0] // N_CORES
    in_maps = [
        {
            "scores": scores[i * rows : (i + 1) * rows],
            "labels": labels[i * rows : (i + 1) * rows],
            "temperature": temperature,
        }
        for i in range(N_CORES)
    ]
    res = run_bass_kernel_spmd(nc, in_maps, list(range(N_CORES)))
    parts = np.stack([res.results[i]["out"][0] for i in range(N_CORES)])
    return combine_partials(parts, temperature[0], scores.shape[0],
                            scores.shape[1])


# revision 5
# speedup vs baseline: 1.1531x; 1.1531x over previous
"""CombinedRankingLoss Trainium2 Bass kernel (concatenated-scan version).

Data-parallel over 8 NeuronCores: each core takes a [1024, 1024] slice of
scores/labels, computes partial sums of the loss components, host combines
the 8 partial vectors into the final scalar.

Math (x = scores/clip(temp), b = labels>0, e = exp(x)):
  ListMLE with labels in {0..4}: for element j with label v>=1,
    T_j = D_v - P_exc,v(j)
  where P_exc,v = exclusive prefix of e*[l==v] along the row and
  D_v = sum of e over labels <= v.  Concatenating the four masked-e
  streams per row-group as [me_4 ++ me_3 ++ me_2 ++ me_1] and running ONE
  exclusive-prefix scan with initial = 1 - S gives, at class-v's segment,
    out = 1 - S + sum_{u>v} E_u + P_exc,v(j) = P_exc,v(j) - (D_v - 1)
  because S = D_v + sum_{u>v} E_u.  That is z_v for every class at once —
  no D-chain, 8 scans instead of 32.  Then uv = m_cat * z (one flat
  multiply; masks disjoint and {0,1}, pad lane 0) and, exactly,
    sum_j ln(1 - uv(j)) = sum_{j labeled} ln T_j
  so one ACT Ln(scale=-1, bias=1) with accum_out per row-group produces
  the per-group lnT sums over all four classes.
  per_list = (L_g - inv_t*Wx_g) / (K_g + eps); host sums over rows.
  Focal/BCE: ur = scob - 2*mpos*scob = x*(1-2b) in bf16;
  pneg = sigmoid(-inv_t*ur); s2 = (1-pneg)^2; F = sum s2*ln(pneg);
  C = sum ln(pneg); Ssco = sum x.  Host combines (A, F, C, Wx, Ssco).

Engine layout: DVE owns the scans, masks (tensor_scalar), and the
tensor_tensor multiplies; ACT does exp (+S accum), scob copy (+Ssco),
K via Sign (+accum), sigmoid/square/ln, and the per-group masked-ln
accums.  The focal chain issues before the scans so its ACT work
overlaps them.  GPSIMD measured far below roofline when interleaved
with this flow; unused.
"""

import numpy as np

import concourse.bass as bass
import concourse.bacc as bacc
import concourse.mybir as mybir
from concourse.tile import TileContext

AL = mybir.AluOpType
AF = mybir.ActivationFunctionType
AX = mybir.AxisListType
F32 = mybir.dt.float32
BF16 = mybir.dt.bfloat16
I32 = mybir.dt.int32

N_CORES = 8
B_FULL = 8192
N = 1024
ROWS_PER_CORE = B_FULL // N_CORES
EPS = 1e-10


def build_nc(rows=ROWS_PER_CORE, n=N, GS=2, time_reps=1):
    P = 128
    G = rows // P
    S_STEPS = G // GS
    W = 4 * n + 1  # concat width incl pad col

    nc = bacc.Bacc("TRN2", target_bir_lowering=False, debug=False)
    d_scores = nc.dram_tensor("scores", [rows, n], F32, kind="ExternalInput")
    d_labels = nc.dram_tensor("labels", [rows, n], I32, kind="ExternalInput")
    d_temp = nc.dram_tensor("temperature", [1], F32, kind="ExternalInput")
    d_out = nc.dram_tensor("out", [1, 8], F32, kind="ExternalOutput")

    sc_re = d_scores.rearrange("(g p) n -> p g n", p=P)
    lb_re = d_labels.rearrange("(g p) n -> p g n", p=P)
    tp_re = d_temp.rearrange("(p a) -> p a", p=1)

    def fl(t):
        return t[:].rearrange("p g n -> p (g n)")

    with TileContext(nc) as tc:
        with (
            tc.tile_pool(name="const", bufs=1) as cpool,
            tc.tile_pool(name="io", bufs=2) as iopool,
            tc.tile_pool(name="wk", bufs=1) as wk,
            tc.tile_pool(name="st", bufs=1) as st,
            tc.tile_pool(name="ps", bufs=1, space="PSUM") as pspool,
        ):
            ones_n = cpool.tile([P, 4 * n], BF16, tag="ones", name="ones")
            nc.vector.memset(ones_n[:], 1.0)
            t_raw = cpool.tile([P, 1], F32, tag="traw", name="traw")
            t_clip = cpool.tile([P, 1], F32, tag="tclip", name="tclip")
            inv_t = cpool.tile([P, 1], F32, tag="invt", name="invt")
            neg_inv_t = cpool.tile([P, 1], F32, tag="ninvt", name="ninvt")
            nc.sync.dma_start(t_raw[:], tp_re[:, :].partition_broadcast(P))
            nc.vector.tensor_scalar(t_clip[:], t_raw[:], 0.1, 5.0, AL.max, AL.min)
            nc.vector.reciprocal(inv_t[:], t_clip[:])
            nc.vector.tensor_scalar(neg_inv_t[:], inv_t[:], -1.0, None, AL.mult)

            tS = st.tile([P, G], F32, tag="S", name="S")
            tL = st.tile([P, G], F32, tag="L", name="L")
            tWx = st.tile([P, G], F32, tag="Wx", name="Wx")
            tK = st.tile([P, G], F32, tag="K", name="K")
            tF = st.tile([P, S_STEPS], F32, tag="F", name="F")
            tC = st.tile([P, S_STEPS], F32, tag="C", name="C")
            tX = st.tile([P, S_STEPS], F32, tag="X", name="X")
            for _t in (tS, tL, tWx, tK, tF, tC, tX):
                nc.vector.memset(_t[:], 0.0)

            from contextlib import nullcontext
            loop_cm = tc.For_i(0, time_reps, 1) if time_reps > 1 else nullcontext()
            with loop_cm:
              for s in range(S_STEPS):
                g0 = s * GS
                sl = slice(g0, g0 + GS)
                sco = iopool.tile([P, GS, n], F32, tag="sco", name="sco")
                lab = iopool.tile([P, GS, n], I32, tag="lab", name="lab")
                nc.sync.dma_start(sco[:], sc_re[:, sl, :])
                nc.sync.dma_start(lab[:], lb_re[:, sl, :])

                labf = wk.tile([P, GS, n], BF16, tag="labf", name="labf")
                e = wk.tile([P, GS, n], BF16, tag="e", name="e")
                m_cat = wk.tile([P, GS, W], BF16, tag="mcat", name="mcat")
                me_cat = wk.tile([P, GS, W], BF16, tag="mecat", name="mecat")
                z = wk.tile([P, GS, W], BF16, tag="z", name="z")
                negD4 = wk.tile([P, GS], F32, tag="negD4", name="negD4")
                scob = wk.tile([P, GS, n], BF16, tag="scob", name="scob")
                mpos = wk.tile([P, GS, n], BF16, tag="mpos", name="mpos")
                w_t = wk.tile([P, GS, n], BF16, tag="w", name="w")
                t1 = wk.tile([P, GS, n], BF16, tag="t1", name="t1")
                pneg = wk.tile([P, GS, n], BF16, tag="pneg", name="pneg")

                nc.vector.tensor_scalar(labf[:], lab[:], 1.0, None, AL.mult)
                # scob first: unblocks the DVE w-chain earliest
                nc.scalar.activation(fl(scob), fl(sco), AF.Copy,
                                     accum_out=tX[:, s : s + 1])
                for gi in range(GS):
                    nc.scalar.activation(
                        e[:, gi, :], sco[:, gi, :], AF.Exp,
                        scale=inv_t[:], accum_out=tS[:, g0 + gi : g0 + gi + 1])
                for gi in range(GS):
                    nc.scalar.activation(
                        pneg[:, gi, :], labf[:, gi, :], AF.Sign,
                        accum_out=tK[:, g0 + gi : g0 + gi + 1])
                # masks into concat segments (class 4 first); pad col -> 0
                nc.vector.memset(m_cat[:, :, 4 * n : W], 0.0)
                for seg, v in enumerate((4, 3, 2, 1)):
                    nc.vector.tensor_scalar(
                        m_cat[:, :, seg * n : (seg + 1) * n], labf[:],
                        float(v), None, AL.is_equal)
                nc.vector.tensor_scalar(fl(mpos), fl(labf), 1.0, None,
                                        AL.is_ge)
                # focal chain (its ACT part overlaps the scans below).
                # Sigmoid-free: with u = inv_t*ur, q = softplus(u) =
                # Ln(1 + Exp(u)) = -ln(pneg); 1-pneg = 1 - Exp(-q);
                # s2 = Square(1 - Exp(-q)).  Every func lives in the
                # natural_log_exp ACT table: zero table switches.
                nc.vector.tensor_tensor(fl(w_t), fl(mpos), fl(scob), AL.mult)
                # Wx_g via ACT copy+accum (keeps the reduce off DVE)
                for gi in range(GS):
                    nc.scalar.activation(
                        pneg[:, gi, :], w_t[:, gi, :], AF.Copy,
                        accum_out=tWx[:, g0 + gi : g0 + gi + 1])
                nc.vector.tensor_scalar(fl(t1), fl(w_t), -2.0, None, AL.mult)
                ur = w_t  # w dead after t1
                nc.vector.tensor_tensor(fl(ur), fl(t1), fl(scob), AL.add)
                eu = pneg
                nc.scalar.activation(fl(eu), fl(ur), AF.Exp, scale=inv_t[:])
                q = t1  # t1 dead after ur; q = softplus(u) = -lnp
                nc.scalar.activation(fl(q), fl(eu), AF.Ln, bias=1.0,
                                     accum_out=tC[:, s : s + 1])
                r = mpos  # mpos dead after w
                nc.scalar.activation(fl(r), fl(q), AF.Exp, scale=-1.0)
                s2 = eu  # eu dead after q
                nc.scalar.activation(fl(s2), fl(r), AF.Square,
                                     scale=-1.0, bias=1.0)
                # me_cat segments = m_seg * e
                for seg in range(4):
                    nc.vector.tensor_tensor(
                        me_cat[:, :, seg * n : (seg + 1) * n],
                        m_cat[:, :, seg * n : (seg + 1) * n], e[:], AL.mult)
                nc.vector.tensor_scalar(negD4[:], tS[:, sl], -1.0, 1.0,
                                        AL.mult, AL.add)
                # one scan per row-group over the 4n concat
                for g in range(GS):
                    nc.vector.tensor_tensor_scan(
                        z[:, g, 1:W], ones_n[:], me_cat[:, g, 0 : 4 * n],
                        negD4[:, g : g + 1], AL.mult, AL.add)
                nc.vector.tensor_copy(
                    z[:, :, 0:1], negD4[:].rearrange("p (g o) -> p g o", o=1))
                # gg here: the focal ACT chain is surely done, DVE mid-flight
                # gg = s2*q = -s2*lnp; host negates via the F sign
                gg = r  # r dead after s2
                nc.vector.tensor_tensor(fl(gg), fl(s2), fl(q), AL.mult)
                nc.vector.tensor_reduce(tF[:, s : s + 1], fl(gg), AX.X, AL.add)
                # uv = m_cat * z (one flat multiply), per-group masked-ln
                nc.vector.tensor_tensor(fl(me_cat), fl(m_cat), fl(z), AL.mult)
                for g in range(GS):
                    nc.scalar.activation(
                        z[:, g, :], me_cat[:, g, :], AF.Ln,
                        scale=-1.0, bias=1.0,
                        accum_out=tL[:, g0 + g : g0 + g + 1])

            # ---- epilogue ----
            ep = st.tile([P, G], F32, tag="ep1", name="ep1")
            ep2 = st.tile([P, G], F32, tag="ep2", name="ep2")
            stat = st.tile([P, 8], F32, tag="stat", name="stat")
            statr = st.tile([P, 8], F32, tag="statr", name="statr")
            nc.vector.memset(stat[:], 0.0)
            nc.vector.tensor_reduce(stat[:, 1:2], tF[:], AX.X, AL.add)
            nc.vector.tensor_reduce(stat[:, 2:3], tC[:], AX.X, AL.add)
            nc.vector.tensor_reduce(stat[:, 3:4], tWx[:], AX.X, AL.add)
            nc.vector.tensor_reduce(stat[:, 4:5], tX[:], AX.X, AL.add)
            nc.vector.tensor_scalar(ep2[:], tWx[:], inv_t[:], None, AL.mult)
            nc.vector.tensor_tensor(ep[:], tL[:], ep2[:], AL.subtract)
            nc.vector.tensor_scalar(ep2[:], tK[:], EPS, None, AL.add)
            nc.vector.reciprocal(ep2[:], ep2[:])
            nc.vector.tensor_tensor(ep[:], ep[:], ep2[:], AL.mult)
            nc.vector.tensor_reduce(stat[:, 0:1], ep[:], AX.X, AL.add)
            ones_f = cpool.tile([P, 1], F32, tag="onesf", name="onesf")
            nc.vector.memset(ones_f[:], 1.0)
            ps_out = pspool.tile([1, 8], F32, tag="psout", name="psout")
            nc.tensor.matmul(ps_out[:], ones_f[:], stat[:], start=True, stop=True)
            nc.vector.tensor_copy(statr[0:1, :], ps_out[:])
            nc.sync.dma_start(d_out[:, :], statr[0:1, :])
    nc.compile()
    return nc


def combine_partials(parts, temp_val, b_full, n):
    """parts: [n_cores, 8] f32 partial sums; cols: A, F, C, Wx, Ssco."""
    parts = np.asarray(parts, dtype=np.float64)
    inv_t = 1.0 / float(np.clip(temp_val, 0.1, 5.0))
    A = parts[:, 0].sum()      # sum of per-row listmle
    F_ = parts[:, 1].sum()     # sum s2*softplus = -sum s2*lnp
    C = parts[:, 2].sum()      # sum softplus = -sum lnp
    Wx = parts[:, 3].sum()     # sum b*scores (bf16)
    Ss = parts[:, 4].sum()     # sum scores
    U = Ss - 2.0 * Wx          # sum scores*(1-2b)
    cnt = float(b_full) * n
    listmle = A / b_full
    ce_sum = C
    focal = (0.25 * F_) / cnt
    sum_x = inv_t * (U + 2.0 * Wx)
    sum_xb = inv_t * Wx
    smooth = (ce_sum + 0.1 * sum_xb - (0.1 / n) * sum_x) / cnt
    total = 0.7 * listmle + 0.3 * focal + 0.1 * smooth
    return np.asarray(total, dtype=np.float32)


_CACHED = {}


def kernel(scores, temperature, labels):
    from concourse.bass_utils import run_bass_kernel_spmd

    scores = np.ascontiguousarray(np.asarray(scores), dtype=np.float32)
    labels = np.ascontiguousarray(np.asarray(labels), dtype=np.int32)
    temperature = np.asarray(temperature, dtype=np.float32).reshape(1)

    key = (scores.shape, labels.shape)
    if key not in _CACHED:
        _CACHED[key] = build_nc(rows=scores.shape[0] // N_CORES,
                                n=scores.shape[1])
    nc = _CACHED[key]

    rows = scores.shape[0] // N_CORES
    in_maps = [
        {
            "scores": scores[i * rows : (i + 1) * rows],
            "labels": labels[i * rows : (i + 1) * rows],
            "temperature": temperature,
        }
        for i in range(N_CORES)
    ]
    res = run_bass_kernel_spmd(nc, in_maps, list(range(N_CORES)))
    parts = np.stack([res.results[i]["out"][0] for i in range(N_CORES)])
    return combine_partials(parts, temperature[0], scores.shape[0],
                            scores.shape[1])


# revision 11
# speedup vs baseline: 1.1554x; 1.0021x over previous
"""CombinedRankingLoss Trainium2 Bass kernel (concatenated-scan version).

Data-parallel over 8 NeuronCores: each core takes a [1024, 1024] slice of
scores/labels, computes partial sums of the loss components, host combines
the 8 partial vectors into the final scalar.

Math (x = scores/clip(temp), b = labels>0, e = exp(x)):
  ListMLE with labels in {0..4}: for element j with label v>=1,
    T_j = D_v - P_exc,v(j)
  where P_exc,v = exclusive prefix of e*[l==v] along the row and
  D_v = sum of e over labels <= v.  Concatenating the four masked-e
  streams per row-group as [me_4 ++ me_3 ++ me_2 ++ me_1] and running ONE
  exclusive-prefix scan with initial = 1 - S gives, at class-v's segment,
    out = 1 - S + sum_{u>v} E_u + P_exc,v(j) = P_exc,v(j) - (D_v - 1)
  because S = D_v + sum_{u>v} E_u.  That is z_v for every class at once —
  no D-chain, 8 scans instead of 32.  Then uv = m_cat * z (one flat
  multiply; masks disjoint and {0,1}, pad lane 0) and, exactly,
    sum_j ln(1 - uv(j)) = sum_{j labeled} ln T_j
  so one ACT Ln(scale=-1, bias=1) with accum_out per row-group produces
  the per-group lnT sums over all four classes.
  per_list = (L_g - inv_t*Wx_g) / (K_g + eps); host sums over rows.
  Focal/BCE: ur = scob - 2*mpos*scob = x*(1-2b) in bf16;
  pneg = sigmoid(-inv_t*ur); s2 = (1-pneg)^2; F = sum s2*ln(pneg);
  C = sum ln(pneg); Ssco = sum x.  Host combines (A, F, C, Wx, Ssco).

Engine layout: DVE owns the scans, masks (tensor_scalar), and the
tensor_tensor multiplies; ACT does exp (+S accum), scob copy (+Ssco),
K via Sign (+accum), sigmoid/square/ln, and the per-group masked-ln
accums.  The focal chain issues before the scans so its ACT work
overlaps them.  GPSIMD measured far below roofline when interleaved
with this flow; unused.
"""

import numpy as np

import concourse.bass as bass
import concourse.bacc as bacc
import concourse.mybir as mybir
from concourse.tile import TileContext

AL = mybir.AluOpType
AF = mybir.ActivationFunctionType
AX = mybir.AxisListType
F32 = mybir.dt.float32
BF16 = mybir.dt.bfloat16
I32 = mybir.dt.int32

N_CORES = 8
B_FULL = 8192
N = 1024
ROWS_PER_CORE = B_FULL // N_CORES
EPS = 1e-10


def build_nc(rows=ROWS_PER_CORE, n=N, GS=2, time_reps=1):
    P = 128
    G = rows // P
    S_STEPS = G // GS
    W = 4 * n + 1  # concat width incl pad col

    nc = bacc.Bacc("TRN2", target_bir_lowering=False, debug=False)
    d_scores = nc.dram_tensor("scores", [rows, n], F32, kind="ExternalInput")
    d_labels = nc.dram_tensor("labels", [rows, n], I32, kind="ExternalInput")
    d_temp = nc.dram_tensor("temperature", [1], F32, kind="ExternalInput")
    d_out = nc.dram_tensor("out", [1, 8], F32, kind="ExternalOutput")

    sc_re = d_scores.rearrange("(g p) n -> p g n", p=P)
    lb_re = d_labels.rearrange("(g p) n -> p g n", p=P)
    tp_re = d_temp.rearrange("(p a) -> p a", p=1)

    def fl(t):
        return t[:].rearrange("p g n -> p (g n)")

    with TileContext(nc) as tc:
        with (
            tc.tile_pool(name="const", bufs=1) as cpool,
            tc.tile_pool(name="io", bufs=2) as iopool,
            tc.tile_pool(name="wk", bufs=1) as wk,
            tc.tile_pool(name="st", bufs=1) as st,
            tc.tile_pool(name="ps", bufs=1, space="PSUM") as pspool,
        ):
            ones_n = cpool.tile([P, 4 * n], BF16, tag="ones", name="ones")
            nc.vector.memset(ones_n[:], 1.0)
            t_raw = cpool.tile([P, 1], F32, tag="traw", name="traw")
            t_clip = cpool.tile([P, 1], F32, tag="tclip", name="tclip")
            inv_t = cpool.tile([P, 1], F32, tag="invt", name="invt")
            neg_inv_t = cpool.tile([P, 1], F32, tag="ninvt", name="ninvt")
            nc.sync.dma_start(t_raw[:], tp_re[:, :].partition_broadcast(P))
            nc.vector.tensor_scalar(t_clip[:], t_raw[:], 0.1, 5.0, AL.max, AL.min)
            nc.vector.reciprocal(inv_t[:], t_clip[:])
            nc.vector.tensor_scalar(neg_inv_t[:], inv_t[:], -1.0, None, AL.mult)

            tS = st.tile([P, G], F32, tag="S", name="S")
            tL = st.tile([P, G], F32, tag="L", name="L")
            tWx = st.tile([P, G], F32, tag="Wx", name="Wx")
            tK = st.tile([P, G], F32, tag="K", name="K")
            tF = st.tile([P, S_STEPS], F32, tag="F", name="F")
            tC = st.tile([P, S_STEPS], F32, tag="C", name="C")
            tX = st.tile([P, S_STEPS], F32, tag="X", name="X")
            for _t in (tS, tL, tWx, tK, tF, tC, tX):
                nc.vector.memset(_t[:], 0.0)

            from contextlib import nullcontext
            loop_cm = tc.For_i(0, time_reps, 1) if time_reps > 1 else nullcontext()
            with loop_cm:
              for s in range(S_STEPS):
                g0 = s * GS
                sl = slice(g0, g0 + GS)
                sco = iopool.tile([P, GS, n], F32, tag="sco", name="sco")
                lab = iopool.tile([P, GS, n], I32, tag="lab", name="lab")
                nc.sync.dma_start(sco[:], sc_re[:, sl, :])
                nc.sync.dma_start(lab[:], lb_re[:, sl, :])

                labf = wk.tile([P, GS, n], BF16, tag="labf", name="labf")
                e = wk.tile([P, GS, n], BF16, tag="e", name="e")
                m_cat = wk.tile([P, GS, W], BF16, tag="mcat", name="mcat")
                me_cat = wk.tile([P, GS, W], BF16, tag="mecat", name="mecat")
                z = wk.tile([P, GS, W], BF16, tag="z", name="z")
                negD4 = wk.tile([P, GS], F32, tag="negD4", name="negD4")
                scob = wk.tile([P, GS, n], BF16, tag="scob", name="scob")
                mpos = wk.tile([P, GS, n], BF16, tag="mpos", name="mpos")
                w_t = wk.tile([P, GS, n], BF16, tag="w", name="w")
                t1 = wk.tile([P, GS, n], BF16, tag="t1", name="t1")
                pneg = wk.tile([P, GS, n], BF16, tag="pneg", name="pneg")

                nc.vector.tensor_scalar(labf[:], lab[:], 1.0, None, AL.mult)
                # scob first: unblocks the DVE w-chain earliest
                nc.scalar.activation(fl(scob), fl(sco), AF.Copy,
                                     accum_out=tX[:, s : s + 1])
                for gi in range(GS):
                    nc.scalar.activation(
                        e[:, gi, :], sco[:, gi, :], AF.Exp,
                        scale=inv_t[:], accum_out=tS[:, g0 + gi : g0 + gi + 1])
                for gi in range(GS):
                    nc.scalar.activation(
                        pneg[:, gi, :], labf[:, gi, :], AF.Sign,
                        accum_out=tK[:, g0 + gi : g0 + gi + 1])
                # masks into concat segments (class 4 first); pad col -> 0
                nc.vector.memset(m_cat[:, :, 4 * n : W], 0.0)
                for seg, v in enumerate((4, 3, 2, 1)):
                    nc.vector.tensor_scalar(
                        m_cat[:, :, seg * n : (seg + 1) * n], labf[:],
                        float(v), None, AL.is_equal)
                nc.vector.tensor_scalar(fl(mpos), fl(labf), 1.0, None,
                                        AL.is_ge)
                # focal chain (its ACT part overlaps the scans below).
                # Sigmoid-free: with u = inv_t*ur, q = softplus(u) =
                # Ln(1 + Exp(u)) = -ln(pneg); 1-pneg = 1 - Exp(-q);
                # s2 = Square(1 - Exp(-q)).  Every func lives in the
                # natural_log_exp ACT table: zero table switches.
                nc.vector.tensor_tensor(fl(w_t), fl(mpos), fl(scob), AL.mult)
                # Wx_g via ACT copy+accum (keeps the reduce off DVE)
                for gi in range(GS):
                    nc.scalar.activation(
                        pneg[:, gi, :], w_t[:, gi, :], AF.Copy,
                        accum_out=tWx[:, g0 + gi : g0 + gi + 1])
                nc.vector.tensor_scalar(fl(t1), fl(w_t), -2.0, None, AL.mult)
                ur = w_t  # w dead after t1
                nc.vector.tensor_tensor(fl(ur), fl(t1), fl(scob), AL.add)
                eu = pneg
                nc.scalar.activation(fl(eu), fl(ur), AF.Exp, scale=inv_t[:])
                q = t1  # t1 dead after ur; q = softplus(u) = -lnp
                nc.scalar.activation(fl(q), fl(eu), AF.Ln, bias=1.0,
                                     accum_out=tC[:, s : s + 1])
                r = mpos  # mpos dead after w
                nc.scalar.activation(fl(r), fl(q), AF.Exp, scale=-1.0)
                s2 = eu  # eu dead after q
                nc.scalar.activation(fl(s2), fl(r), AF.Square,
                                     scale=-1.0, bias=1.0)
                # me_cat segments = m_seg * e
                for seg in range(4):
                    nc.vector.tensor_tensor(
                        me_cat[:, :, seg * n : (seg + 1) * n],
                        m_cat[:, :, seg * n : (seg + 1) * n], e[:], AL.mult)
                nc.vector.tensor_scalar(negD4[:], tS[:, sl], -1.0, 1.0,
                                        AL.mult, AL.add)
                # one scan per row-group over the 4n concat
                for g in range(GS):
                    nc.vector.tensor_tensor_scan(
                        z[:, g, 1:W], ones_n[:], me_cat[:, g, 0 : 4 * n],
                        negD4[:, g : g + 1], AL.mult, AL.add)
                nc.vector.tensor_copy(
                    z[:, :, 0:1], negD4[:].rearrange("p (g o) -> p g o", o=1))
                # gg here: the focal ACT chain is surely done, DVE mid-flight
                # gg = s2*q = -s2*lnp; host negates via the F sign
                gg = r  # r dead after s2
                nc.vector.tensor_tensor(fl(gg), fl(s2), fl(q), AL.mult)
                nc.vector.tensor_reduce(tF[:, s : s + 1], fl(gg), AX.X, AL.add)
                # uv = m_cat * z (one flat multiply), per-group masked-ln
                nc.vector.tensor_tensor(fl(me_cat), fl(m_cat), fl(z), AL.mult)
                for g in range(GS):
                    nc.scalar.activation(
                        z[:, g, :], me_cat[:, g, :], AF.Ln,
                        scale=-1.0, bias=1.0,
                        accum_out=tL[:, g0 + g : g0 + g + 1])

            # ---- epilogue ----
            ep = st.tile([P, G], F32, tag="ep1", name="ep1")
            ep2 = st.tile([P, G], F32, tag="ep2", name="ep2")
            stat = st.tile([P, 8], F32, tag="stat", name="stat")
            statr = st.tile([P, 8], F32, tag="statr", name="statr")
            nc.vector.memset(stat[:], 0.0)
            nc.vector.tensor_reduce(stat[:, 1:2], tF[:], AX.X, AL.add)
            nc.vector.tensor_reduce(stat[:, 2:3], tC[:], AX.X, AL.add)
            nc.vector.tensor_reduce(stat[:, 3:4], tWx[:], AX.X, AL.add)
            nc.vector.tensor_reduce(stat[:, 4:5], tX[:], AX.X, AL.add)
            nc.vector.tensor_scalar(ep2[:], tWx[:], inv_t[:], None, AL.mult)
            nc.vector.tensor_tensor(ep[:], tL[:], ep2[:], AL.subtract)
            nc.vector.tensor_scalar(ep2[:], tK[:], EPS, None, AL.add)
            nc.vector.reciprocal(ep2[:], ep2[:])
            nc.vector.tensor_tensor(ep[:], ep[:], ep2[:], AL.mult)
            nc.vector.tensor_reduce(stat[:, 0:1], ep[:], AX.X, AL.add)
            ones_f = cpool.tile([P, 1], F32, tag="onesf", name="onesf")
            nc.vector.memset(ones_f[:], 1.0)
            ps_out = pspool.tile([1, 8], F32, tag="psout", name="psout")
            nc.tensor.matmul(ps_out[:], ones_f[:], stat[:], start=True, stop=True)
            nc.vector.tensor_copy(statr[0:1, :], ps_out[:])
            nc.sync.dma_start(d_out[:, :], statr[0:1, :])
    nc.compile()
    return nc


def combine_partials(parts, temp_val, b_full, n):
    """parts: [n_cores, 8] f32 partial sums; cols: A, F, C, Wx, Ssco."""
    parts = np.asarray(parts, dtype=np.float64)
    inv_t = 1.0 / float(np.clip(temp_val, 0.1, 5.0))
    A = parts[:, 0].sum()      # sum of per-row listmle
    F_ = parts[:, 1].sum()     # sum s2*softplus = -sum s2*lnp
    C = parts[:, 2].sum()      # sum softplus = -sum lnp
    Wx = parts[:, 3].sum()     # sum b*scores (bf16)
    Ss = parts[:, 4].sum()     # sum scores
    U = Ss - 2.0 * Wx          # sum scores*(1-2b)
    cnt = float(b_full) * n
    listmle = A / b_full
    ce_sum = C
    focal = (0.25 * F_) / cnt
    sum_x = inv_t * (U + 2.0 * Wx)
    sum_xb = inv_t * Wx
    smooth = (ce_sum + 0.1 * sum_xb - (0.1 / n) * sum_x) / cnt
    total = 0.7 * listmle + 0.3 * focal + 0.1 * smooth
    return np.asarray(total, dtype=np.float32)


_CACHED = {}


def kernel(scores, temperature, labels):
    from concourse.bass_utils import run_bass_kernel_spmd

    scores = np.ascontiguousarray(np.asarray(scores), dtype=np.float32)
    labels = np.ascontiguousarray(np.asarray(labels), dtype=np.int32)
    temperature = np.asarray(temperature, dtype=np.float32).reshape(1)

    key = (scores.shape, labels.shape)
    if key not in _CACHED:
        _CACHED[key] = build_nc(rows=scores.shape[0] // N_CORES,
                                n=scores.shape[1])
    nc = _CACHED[key]

    rows = scores.shape[0] // N_CORES
    in_maps = [
        {
            "scores": scores[i * rows : (i + 1) * rows],
            "labels": labels[i * rows : (i + 1) * rows],
            "temperature": temperature,
        }
        for i in range(N_CORES)
    ]
    res = run_bass_kernel_spmd(nc, in_maps, list(range(N_CORES)))
    parts = np.stack([res.results[i]["out"][0] for i in range(N_CORES)])
    return combine_partials(parts, temperature[0], scores.shape[0],
                            scores.shape[1])
